# revision 1
# baseline (speedup 1.0000x reference)
"""ExpFloatLinear kernel for Trainium2 (8 NeuronCores, SPMD).

Computes out = qd(qd(x) @ qd(W^T) + qd(bias)) where
qd(t) = 2^round(log2|t|)  (sign dropped, clamp to [-128,127]).

Implementation notes:
- qd on fp32 is two DVE tensor_scalar ops: u = t * fp32(sqrt2) (float mult),
  then bits(u) & 0x7F800000 on the uint32 bitcast view (bitvec ops require
  an int dtype; OR with 0.0 fills the float-only scalar1 slot so the int
  mask can ride in scalar2). Multiplying by sqrt2 bumps the exponent
  exactly when mantissa >= sqrt(2): both the exact threshold and the
  fp32-rounded one fall strictly between the same pair of representable
  mantissas, so this equals 2^round(log2|t|) for every normal fp32 input.
- Quantized values are exact powers of two, so the matmul runs in fp8
  (e4m3) with the DoubleRow perf mode at ~1.4x the bf16 PE rate. Powers
  of two in [2^-9, 2^7] are exact in e4m3; folding a power-of-two scale
  into the quant constant (x * 2^4, w * 2^13) brings all but a vanishing
  tail of the operands into that window (out-of-range-small values cast
  to 0; their contribution to the 4096-term dot products is < 1e-6
  relative, far below the final re-quantization's rounding granularity).
  The matching descale 2^-17 folds into the epilogue quant constant.
- Sharding: rows of x split across the 8 cores (1024 rows each); weight
  and bias replicated. Each core computes its [1024, 4096] output slab.
"""

import numpy as np

P = 128
SQRT2 = float(np.uint32(0x3FB504F3).view(np.float32))  # fp32 nearest sqrt(2)
MASK = 0x7F800000
SCALE_X = 4
SCALE_W = 13
QS_X = SQRT2 * 2.0**SCALE_X
QS_W = SQRT2 * 2.0**SCALE_W
QS_M = SQRT2 * 2.0 ** -(SCALE_X + SCALE_W)

N_CORES = 8
FULL_M, FULL_K, FULL_N = 8192, 4096, 4096

_compiled = {}


def _build(M_SHARD, K, N, n_cores, loops=1):
    from contextlib import ExitStack

    import concourse.mybir as mybir
    import concourse.tile as tile
    from concourse import bacc

    f32 = mybir.dt.float32
    bf16 = mybir.dt.bfloat16
    fp8 = mybir.dt.float8e4
    u32 = mybir.dt.uint32
    MUL = mybir.AluOpType.mult
    AND = mybir.AluOpType.bitwise_and
    ORR = mybir.AluOpType.bitwise_or
    DR = mybir.MatmulPerfMode.DoubleRow

    MT = M_SHARD // P      # m-tiles of 128 rows
    KO = K // P            # k-tiles of 128
    KOP = KO // 2          # DoubleRow k-pair tiles
    NCHUNK = 512
    NCH = N // NCHUNK      # n-chunks
    NS = NCHUNK // P       # 128-row subtiles per n-chunk
    KH = K // 2            # half-K staging to save SBUF
    KOH = KO // 2

    nc = bacc.Bacc(
        "TRN2",
        target_bir_lowering=False,
        debug=False,
        num_devices=n_cores,
    )

    x = nc.dram_tensor("x", [M_SHARD, K], f32, kind="ExternalInput").ap()
    w = nc.dram_tensor("w", [N, K], f32, kind="ExternalInput").ap()
    b = nc.dram_tensor("b", [N], f32, kind="ExternalInput").ap()
    out = nc.dram_tensor("out", [M_SHARD, N], f32, kind="ExternalOutput").ap()

    with ExitStack() as ctx:
        tc = ctx.enter_context(tile.TileContext(nc))

        xq_pool = ctx.enter_context(tc.tile_pool(name="xq", bufs=1))
        wq_pool = ctx.enter_context(tc.tile_pool(name="wq", bufs=2))
        stage32 = ctx.enter_context(tc.tile_pool(name="stage32", bufs=6))
        stage16 = ctx.enter_context(tc.tile_pool(name="stage16", bufs=6))
        stageT = ctx.enter_context(tc.tile_pool(name="stageT", bufs=4))
        bias_pool = ctx.enter_context(tc.tile_pool(name="bias", bufs=1))
        out_pool = ctx.enter_context(tc.tile_pool(name="outp", bufs=6))
        psum_pool = ctx.enter_context(
            tc.tile_pool(name="psum", bufs=6, space="PSUM")
        )

        def qd_ops(ap, qscale):
            """ap = qd(ap) * 2^s in place, where qscale = sqrt2 * 2^s."""
            nc.vector.tensor_scalar(ap, ap, qscale, None, MUL)
            apu = ap.bitcast(u32)
            nc.vector.tensor_scalar(apu, apu, 0.0, MASK, ORR, AND)

        # quant + cast + transpose one [128, KH] block of natural rows into
        # the bf16 staging tile tT [128(ki), KO, 128]
        def prep_rows(src_rows, tT, kh, qscale):
            t = stage32.tile([P, KH], f32, tag="stage32")
            nc.sync.dma_start(t, src_rows)
            qd_ops(t[:], qscale)
            tb = stage16.tile([P, KH], bf16, tag="stage16")
            nc.any.tensor_copy(out=tb[:], in_=t[:])  # exact: mantissa is zero
            nc.sync.dma_start(
                tT[:, kh * KOH : (kh + 1) * KOH, :], tb, transpose=True
            )

        def body():
            # bias: load to partition 0, replicate by doubling, quantize
            bias_t = bias_pool.tile([P, N], f32, tag="bias")
            nc.sync.dma_start(bias_t[0:1, :], b[None, :])
            p = 1
            while p < P:
                nc.sync.dma_start(bias_t[p : 2 * p, :], bias_t[0:p, :])
                p *= 2
            qd_ops(bias_t[:], SQRT2)

            # x: quantized+scaled, K-major, fp8, resident
            # xq8[ki, mt, kop, j, m] = qd(x)[mt*128+m, (2*kop+j)*128+ki] * 2^SX
            xq8 = xq_pool.tile([P, MT, KOP, 2, P], fp8, tag="xq8")
            for mt in range(MT):
                tT = stageT.tile([P, KO, P], bf16, tag="stageT")
                for kh in range(2):
                    prep_rows(
                        x[mt * P : (mt + 1) * P, kh * KH : (kh + 1) * KH],
                        tT,
                        kh,
                        QS_X,
                    )
                nc.any.tensor_copy(
                    out=xq8[:, mt],
                    in_=tT[:].rearrange("p (kop j) m -> p kop j m", j=2),
                )

            # main loop over n-chunks of 512 output columns
            for nci in range(NCH):
                wq8 = wq_pool.tile([P, KOP, 2, NCHUNK], fp8, tag="wq8")
                for ns in range(NS):
                    r0 = nci * NCHUNK + ns * P
                    tT = stageT.tile([P, KO, P], bf16, tag="stageT")
                    for kh in range(2):
                        prep_rows(
                            w[r0 : r0 + P, kh * KH : (kh + 1) * KH], tT, kh, QS_W
                        )
                    nc.any.tensor_copy(
                        out=wq8[:, :, :, ns * P : (ns + 1) * P],
                        in_=tT[:].rearrange("p (kop j) m -> p kop j m", j=2),
                    )
                for mt in range(MT):
                    ps = psum_pool.tile([P, NCHUNK], f32, tag="ps")
                    for kop in range(KOP):
                        nc.tensor.matmul(
                            ps,
                            xq8[:, mt, kop],
                            wq8[:, kop],
                            start=(kop == 0),
                            stop=(kop == KOP - 1),
                            perf_mode=DR,
                        )
                    o = out_pool.tile([P, NCHUNK], f32, tag="o")
                    nc.vector.tensor_scalar(o[:], ps[:], QS_M, None, MUL)
                    ou = o.bitcast(u32)
                    nc.vector.tensor_scalar(ou, ou, 0.0, MASK, ORR, AND)
                    nc.vector.tensor_tensor(
                        o[:],
                        o[:],
                        bias_t[:, nci * NCHUNK : (nci + 1) * NCHUNK],
                        mybir.AluOpType.add,
                    )
                    nc.vector.tensor_scalar(o[:], o[:], SQRT2, None, MUL)
                    nc.vector.tensor_scalar(ou, ou, 0.0, MASK, ORR, AND)
                    nc.sync.dma_start(
                        out[
                            mt * P : (mt + 1) * P,
                            nci * NCHUNK : (nci + 1) * NCHUNK,
                        ],
                        o,
                    )

        # loops>1 is used only for benchmarking: repeats serialize on tile
        # reuse, so T(loops)-T(1) isolates on-device execution time from
        # upload/dispatch overheads.
        for _ in range(loops):
            body()

    nc.compile()
    return nc


def _get_compiled(M_SHARD, K, N, n_cores, loops=1):
    key = (M_SHARD, K, N, n_cores, loops)
    if key not in _compiled:
        _compiled[key] = _build(M_SHARD, K, N, n_cores, loops)
    return _compiled[key]


def kernel(x, weight, bias):
    from concourse.bass_utils import run_bass_kernel_spmd

    x = np.ascontiguousarray(x, dtype=np.float32)
    weight = np.ascontiguousarray(weight, dtype=np.float32)
    bias = np.ascontiguousarray(bias, dtype=np.float32)
    M, K = x.shape
    N = weight.shape[0]
    assert (M, K, N) == (FULL_M, FULL_K, FULL_N), (M, K, N)

    m_shard = M // N_CORES
    nc = _get_compiled(m_shard, K, N, N_CORES)

    in_maps = [
        {
            "x": x[c * m_shard : (c + 1) * m_shard],
            "w": weight,
            "b": bias,
        }
        for c in range(N_CORES)
    ]
    res = run_bass_kernel_spmd(nc, in_maps, core_ids=list(range(N_CORES)))
    return np.concatenate([r["out"] for r in res.results], axis=0)

